# revision 20
# baseline (speedup 1.0000x reference)
"""Trainium2 Bass kernel for agent-attention (AAGA): 8-core data-parallel over batch.

Math (per batch b):
  qkv = x @ W_qkv + b_qkv ; q,k,v = split(qkv)
  ag  = agent @ W_agent + b_agent ; q_agent,k_agent = split(ag)
  attn1 = softmax(q_agent @ k^T * s)        # [K, N]
  va    = (attn1 @ v) @ W_fc1 + b_fc1       # [K, d]
  attn2 = softmax(q @ k_agent^T * s)        # [N, K]
  out   = (attn2 @ va) @ W_fc2 + b_fc2 + x  # [N, d]

Host-side algebraic folds (everything not involving x is an input):
  q_agent/k_agent computed on host; q,k,v never materialized on device.
  S1^T = x @ (W_k@q_agent^T): b_k drops out of the softmax (shift invariance).
  va-chain: attn1 rows sum to 1, so all later biases fold into a single
       constant row bbig = (b_v@W_fc1+b_fc1)@W_fc2 + b_fc2 ADDED ON HOST.
  Device vaF[k,:] = [ (ec2/s1)[k] * (expS1^T x)[k,:] @ Wbig | ec2[k] ],
  with Wbig = W_v@W_fc1@W_fc2. Then y = expS2^T @ vaF gives
  y[:, :D] = attn2-numerator combination and y[:, D] = s2 (denominator).
  Host epilogue: out = y[:, :D]/y[:, D] + bbig + x  (exact fp32).

DMA/engine regime (cost model): each DMACopy costs ~625ns on a single
serialized HWDGE queue (SP/Act) or ~1us on the Pool engine via SWDGE;
transfers serialize on DMA_ENGINES at 360GB/s with a 2x penalty for
runs <512B. GPSIMD cannot touch PSUM, so all PSUM->SBUF casts go on
DVE+Act, batched 3-4 token-tiles per instruction via multi-bank PSUM
tiles, alternating two PSUM pools so matmul and copy pipeline.
"""

import numpy as np
import ml_dtypes

B, N, D, K = 8, 4096, 256, 64
E = D + 1          # ones-column appended
P = 128
NT = N // P        # 32 token tiles
DS = D // P        # 2 contraction subtiles
W = 512            # free-dim chunk for S2^T
NC2 = N // W       # 8 chunks

# input streaming chunks (in token tiles); small first chunk starts compute
# early, small last chunk cuts the avx tail
XCHUNKS = [4, 8, 8, 8, 4]
# S1 slabs (in token tiles); small last slab shortens the avx tail
SLABS = [4, 8, 8, 8, 4]
# y-phase: 2-tile PSUM groups in a 4-deep ring (ring latency off the
# critical path); one output DMA per 4 tiles, alternating SP/Pool issue
YGROUPS = [2] * 15 + [1, 1]

_BF16 = ml_dtypes.bfloat16
_FP8 = ml_dtypes.float8_e4m3

_CACHE = {}


def _build_nc():
    import concourse.bass as bass
    import concourse.tile as tile
    from concourse import bacc, mybir

    f32 = mybir.dt.float32
    bf16 = mybir.dt.bfloat16
    fp8 = mybir.dt.float8e4
    Exp = mybir.ActivationFunctionType.Exp
    DR = mybir.MatmulPerfMode.DoubleRow
    Copy = mybir.ActivationFunctionType.Copy
    ts = bass.ts

    nc = bacc.Bacc("TRN2", target_bir_lowering=False, debug=False)

    xT_d = nc.declare_dram_parameter("xT", [P, DS, N], fp8, isOutput=False)
    xe_d = nc.declare_dram_parameter("xe", [P, NT, E], fp8, isOutput=False)
    wkq8_d = nc.declare_dram_parameter("wkq8", [P, DS, 2 * K], fp8, isOutput=False)
    wcombo_d = nc.declare_dram_parameter("wcombo", [P, DS, D], bf16, isOutput=False)
    FC = 1 + K         # [ec2 | I64]
    fcombo_d = nc.declare_dram_parameter("fcombo", [K, FC], bf16, isOutput=False)
    ye_d = nc.declare_dram_parameter("ye", [P, NT, E], fp8, isOutput=True)

    with tile.TileContext(nc) as tc:
        with (
            tc.tile_pool(name="sb", bufs=1) as sb,
            tc.tile_pool(name="yout", bufs=8) as yout,
        ):
            # ---------------- input DMAs ----------------
            wkq8 = sb.tile([P, DS, 2 * K], fp8)
            nc.gpsimd.dma_start(out=wkq8, in_=wkq8_d[:, :, :])
            wk = wkq8[:, :, 0:K]
            wq = wkq8[:, :, K : 2 * K]

            xT = sb.tile([P, DS, N], fp8)
            xe = sb.tile([P, NT, E], fp8)
            t0 = 0
            for ci, ct in enumerate(XCHUNKS):
                nc.sync.dma_start(
                    out=xT[:, :, P * t0 : P * (t0 + ct)],
                    in_=xT_d[:, :, P * t0 : P * (t0 + ct)],
                )
                nc.gpsimd.dma_start(
                    out=xe[:, t0 : t0 + ct, :], in_=xe_d[:, t0 : t0 + ct, :]
                )
                if ci == 1:
                    wcombo = sb.tile([P, DS, D], bf16)
                    nc.sync.dma_start(out=wcombo, in_=wcombo_d[:, :, :])
                    fcombo = sb.tile([K, FC], bf16)
                    nc.sync.dma_start(out=fcombo, in_=fcombo_d[:, :])
                t0 += ct
            bias2 = fcombo[:, 0:1]                  # c2*scale - ln(16)
            ident = fcombo[:, 1 : 1 + K]            # I64 bf16

            expS1 = sb.tile([P, NT, K], fp8)    # token-major exp(S1), /16-shifted
            sh1 = sb.tile([P, 1], f32)
            nc.vector.memset(sh1, -2.772588722239781)   # -ln(16): keeps exp < 240 (fp8 max)
            # dummy exp: pulls the 1.3us LoadActFuncSet into the DMA head
            warm = sb.tile([P, 1], f32)
            nc.scalar.activation(warm, sh1, Exp)
            expS2 = sb.tile([K, NC2, W], bf16)          # agent-major exp(S2)

            # vaF holder; col D = 1.0 so y col D = s2 (c2 folded into bias2)
            vaF = sb.tile([K, E], bf16)
            nc.vector.memset(vaF[:, D:E], 1.0)

            # ---- phase 1: S1 slabs + free-axis-paired S2 chunks ----
            # deep PSUM rings so matmul(n+1) never waits exp(n); scoped so the
            # y phase can reuse all 8 banks afterwards
            with (
                tc.tile_pool(name="s1p", bufs=3, space="PSUM") as s1p,  # 3 banks
                tc.tile_pool(name="s2p", bufs=2, space="PSUM") as s2p,  # 4 banks
                tc.tile_pool(name="pX", bufs=1, space="PSUM") as pX,    # 1 bank
            ):
                avx_ps = pX.tile([K, E], f32, tag="pX")
                nslab = len(SLABS)
                sstart = [sum(SLABS[:i]) for i in range(nslab)]

                def s1_slab(b):
                    t0, sl = sstart[b], SLABS[b]
                    ps = s1p.tile([P, sl, K], f32, tag="s1p")
                    for j in range(sl):
                        t = t0 + j
                        # DoubleRow: 2 fp8 weights/cell -> 256-contraction in one mm
                        nc.tensor.matmul(
                            ps[:, j, :], xT[:, :, ts(t, P)], wk,
                            start=True, stop=True, perf_mode=DR,
                        )
                    nc.scalar.activation(
                        expS1[:, t0 : t0 + sl, :], ps, Exp,
                        scale=float(D ** -0.5), bias=sh1,
                    )
                    for j in range(sl // 2):
                        u = t0 // 2 + j
                        nc.tensor.matmul(
                            avx_ps, expS1[:, 2 * u : 2 * u + 2, :],
                            xe[:, 2 * u : 2 * u + 2, :],
                            start=(u == 0), stop=(u == NT // 2 - 1), perf_mode=DR,
                        )

                def s2_pair(h):
                    # two 512-token chunks in two PSUM banks -> one exp, free 1024
                    p2 = s2p.tile([K, 2, W], f32, tag="s2p")
                    for g in range(2):
                        c = 2 * h + g
                        nc.tensor.matmul(
                            p2[:, g, :], wq, xT[:, :, ts(c, W)],
                            start=True, stop=True, perf_mode=DR,
                        )
                    nc.scalar.activation(
                        expS2[:, 2 * h : 2 * h + 2, :], p2, Exp,
                        scale=float(D ** -0.5), bias=bias2,
                    )

                for b in range(nslab):
                    s1_slab(b)
                # S2 logits only feed the y phase; running them after the S1
                # stream keeps Act free so expS1 (which gates avx/vaF) never
                # queues behind a 1us S2 exp. They overlap the vaF chain.
                for h in range(NC2 // 2):
                    s2_pair(h)

                # ---- vaF[:, :D] = (avx/s1 @ Wbig); c2 lives in the exp bias ----
                rec1 = sb.tile([K, 1], f32)
                nc.vector.reciprocal(rec1, avx_ps[:, D:E])
                avx_s = sb.tile([K, D], bf16)
                avxT = sb.tile([P, DS, K], bf16)
                tp = s1p.tile([P, DS, K], bf16, tag="s1p")
                for s in range(DS):
                    # per-half scale then transpose: transpose s starts as soon
                    # as its half of avx_s is written
                    nc.vector.tensor_scalar_mul(
                        avx_s[:, ts(s, P)], avx_ps[:, ts(s, P)], rec1
                    )
                    nc.tensor.transpose(tp[:, s, :], avx_s[:, ts(s, P)], ident)
                nc.vector.tensor_copy(avxT, tp)    # bf16: DVE 2x mode
                vf_ps = s1p.tile([K, D], f32, tag="s1p")
                for s in range(DS):
                    nc.tensor.matmul(
                        vf_ps, avxT[:, s, :], wcombo[:, s, :],
                        start=(s == 0), stop=(s == DS - 1),
                    )
                nc.vector.tensor_copy(vaF[:, 0:D], vf_ps)

            # ---- y_ext[n, :] = sum_k expS2[k,n] * vaF_ext[k, :] ----
            # col D of vaF_ext is ec2, so col D of y_ext = s2. 2-tile groups in
            # a bufs=4 PSUM ring so the copy->matmul ring latency is amortized
            # 4-deep; casts alternate Act/DVE; DMA per 4 tiles, SP/Pool alternating.
            with tc.tile_pool(name="ypool", bufs=4, space="PSUM") as ypool:
                g0 = 0
                y_sb = None
                for gi, gsz in enumerate(YGROUPS):
                    yp = ypool.tile([P, 2, W], f32, tag="ypool")
                    solo = gi >= len(YGROUPS) - 2
                    unit = 1 if solo else (3 if gi >= 12 else 4)
                    u0 = gi - (gi % unit)
                    if gi % unit == 0:
                        ysz = sum(YGROUPS[u0 : u0 + unit])
                        y_sb = yout.tile([P, ysz, E], fp8, tag="ysb")
                        ysb0 = g0
                    for j in range(gsz):
                        t = g0 + j
                        nc.tensor.matmul(
                            yp[:, j, 0:E],
                            expS2[:, t // 4, ts(t % 4, P)],
                            vaF, start=True, stop=True,
                        )
                    dst = y_sb[:, g0 - ysb0 : g0 - ysb0 + gsz, :]
                    if gi % 2 == 0:
                        nc.scalar.activation(dst, yp[:, 0:gsz, 0:E], Copy)
                    else:
                        nc.vector.tensor_copy(dst, yp[:, 0:gsz, 0:E])
                    if gi % unit == unit - 1 or gi == len(YGROUPS) - 1:
                        qn = g0 + gsz - ysb0
                        nc.sync.dma_start(
                            out=ye_d[:, ysb0 : ysb0 + qn, :], in_=y_sb[:, 0:qn, :]
                        )
                    g0 += gsz

    nc.compile()
    return nc


def _get_nc():
    if "nc" not in _CACHE:
        _CACHE["nc"] = _build_nc()
    return _CACHE["nc"]


def _prepare_in_maps(agent, x, W_qkv, b_qkv, W_agent, b_agent, W_fc1, b_fc1, W_fc2, b_fc2):
    # ---- host folds (float64 for stability, cast down at the end) ----
    agent64 = np.asarray(agent, np.float64)
    Wqkv64 = np.asarray(W_qkv, np.float64)
    bqkv64 = np.asarray(b_qkv, np.float64)
    Wag64 = np.asarray(W_agent, np.float64)
    bag64 = np.asarray(b_agent, np.float64)
    Wf1 = np.asarray(W_fc1, np.float64)
    bf1 = np.asarray(b_fc1, np.float64)
    Wf2 = np.asarray(W_fc2, np.float64)
    bf2 = np.asarray(b_fc2, np.float64)

    ag = agent64 @ Wag64 + bag64
    q_agent, k_agent = ag[:, :D], ag[:, D:]
    W_q, W_k, W_v = Wqkv64[:, :D], Wqkv64[:, D : 2 * D], Wqkv64[:, 2 * D :]
    b_q, b_v = bqkv64[:D], bqkv64[2 * D :]

    wk_f = W_k @ q_agent.T                      # [D, K]
    wq_f = W_q @ k_agent.T                      # [D, K]
    c2_f = (D ** -0.5) * (k_agent @ b_q)        # [K]
    ec2_f = np.exp(c2_f)                        # [K]
    Wbig = W_v @ Wf1 @ Wf2                      # [D, D]
    bbig = (b_v @ Wf1 + bf1) @ Wf2 + bf2        # [D], added on host

    # [D, D] -> [P, DS, D] with d = s*128 + p
    wcombo_b = np.ascontiguousarray(
        Wbig.reshape(DS, P, D).transpose(1, 0, 2)
    ).astype(_BF16)
    wkq8 = np.concatenate([wk_f, wq_f], axis=1).reshape(DS, P, 2 * K)
    wkq8 = np.ascontiguousarray(wkq8.transpose(1, 0, 2)).astype(_FP8)
    fcombo = np.zeros((K, 1 + K), np.float32)
    fcombo[:, 0] = c2_f - 2.772588722239781
    fcombo[:, 1 :] = np.eye(K)
    fcombo = np.ascontiguousarray(fcombo).astype(_BF16)

    x32 = np.asarray(x, np.float32)
    # xe pack: [B, N, E] -> [B, P, NT, E], token = t*128 + p
    xb = np.ones((B, N, E), _FP8)
    xb[:, :, :D] = x32.astype(_FP8)
    xeb = np.ascontiguousarray(xb.reshape(B, NT, P, E).transpose(0, 2, 1, 3))
    # xT pack: [B, D, N] -> [B, P, DS, N], d = s*128 + p
    xTb = x32.transpose(0, 2, 1).reshape(B, DS, P, N)
    xTb = np.ascontiguousarray(xTb.transpose(0, 2, 1, 3)).astype(_FP8)

    in_maps = [
        {
            "xT": xTb[i],
            "xe": xeb[i],
            "wkq8": wkq8,
            "wcombo": wcombo_b,
            "fcombo": fcombo,
        }
        for i in range(B)
    ]

    return in_maps, x32, bbig.astype(np.float32)


def kernel(**inputs):
    from concourse.bass_utils import run_bass_kernel_spmd

    in_maps, x32, bbig = _prepare_in_maps(**inputs)
    nc = _get_nc()
    res_obj = run_bass_kernel_spmd(nc, in_maps, core_ids=list(range(B)))
    _CACHE["last_results"] = res_obj
    res = res_obj.results

    # ye [P, NT, E] -> [N, E] with token = t*128 + p
    ye = np.stack(
        [np.asarray(res[i]["ye"]).transpose(1, 0, 2).reshape(N, E) for i in range(B)]
    ).astype(np.float32)
    out = ye[:, :, :D] / ye[:, :, D:E] + bbig[None, None, :] + x32
    return out.astype(np.float32)
